# revision 12
# baseline (speedup 1.0000x reference)
"""Trainium2 Bass kernel for nn_AttentiveAtlasEncoder (vq_codebook).

Pure data-parallel over 8 NeuronCores; each core runs B/8 = 125,000 points
(padded to 62 x 2048-point megatiles).

Per-core pipeline, 3 ACT-table phases (gelu / exp / gelu):
  P1: feature MLP 2->32->32 and fused val+score projection as block-diag x4
      matmuls in transposed layout [rows, 512]; PE thin-transposes the
      [20, 512] (v,scores) result into points-on-partitions chunks of 128.
  P2: softmax routing, c_bar, v_local, 63 codebook distances, segmented
      argmin (eq + reverse-iota), z_q gather via gpsimd indirect_copy,
      VQ loss, blends -- all as large strided DVE ops batched over chunks.
  P3: structure-filter MLP (2->2->2, shared weights) on per-chart residuals
      with immediate-scalar DVE ops + batched gelu.

Outputs are partition-major SBUF dumps [128, n_ch*w]; the host unpermutes
(point = 128*chunk + partition) and slices off padding. vq_loss is emitted
as a per-point partial and summed on host (the cross-core all-reduce).
"""

import sys

sys.path.insert(0, "/opt/trn_rl_repo")

from contextlib import ExitStack

import numpy as np

import concourse.bass as bass  # noqa: F401  (AP types come through tile/bacc)
import concourse.tile as tile
from concourse import bacc, mybir
from concourse.bass_utils import run_bass_kernel_spmd

F32 = mybir.dt.float32
I32 = mybir.dt.int32
U16 = mybir.dt.uint16
AF = mybir.ActivationFunctionType
OP = mybir.AluOpType
AX = mybir.AxisListType

B, IN, H, D, NC_, NK = 1_000_000, 2, 32, 2, 3, 21
N_CORES = 8
B_CORE = B // N_CORES
MT = 2048                        # megatile points (4 groups x 512)
CH_MEGA = 16                     # 128-point chunks per megatile
MEGA_PER_HYPER = 2
CH_H = CH_MEGA * MEGA_PER_HYPER


def _hypers(n_mega):
    out, m = [], 0
    while m < n_mega:
        n = min(MEGA_PER_HYPER, n_mega - m)
        out.append((m, n))
        m += n
    return out


def build_kernel(wts, n_mega):
    """wts: dict with CMAT [3,2], SFW1/SFW2 [2,2], SFB1/SFB2 [2] numpy."""
    n_ch = n_mega * CH_MEGA
    CMAT, SFW1, SFW2 = wts["CMAT"], wts["SFW1"], wts["SFW2"]
    SFB1, SFB2 = wts["SFB1"], wts["SFB2"]

    nc = bacc.Bacc(
        "TRN2", target_bir_lowering=False, debug=False, num_devices=N_CORES
    )

    xT = nc.dram_tensor("xT", [n_mega, 8, 512], F32, kind="ExternalInput").ap()

    def cin(name, shape):
        return nc.dram_tensor(name, shape, F32, kind="ExternalInput").ap()

    s1, s2, s3 = cin("s1", [8, 128]), cin("s2", [128, 128]), cin("s3", [128, 20])
    b1v, b2v = cin("b1v", [128, 1]), cin("b2v", [128, 1])
    bias20, eye20 = cin("bias20", [20, 1]), cin("eye20", [20, 20])
    cb0T, cb1T = cin("cb0T", [128, 63]), cin("cb1T", [128, 63])
    riota, c21 = cin("riota", [128, 63]), cin("c21", [128, 3])
    cbg, chb3 = cin("cbg", [128, 126]), cin("chb3", [128, CH_H])
    sfb1v = cin("sfb1v", [128, 2])
    c012 = cin("c012", [128, 3])

    def out_t(name, w, dt=F32):
        return nc.dram_tensor(name, [128, n_ch * w], dt,
                              kind="ExternalOutput").ap()

    o_router, o_cbar, o_zgeo = out_t("o_router", 3), out_t("o_cbar", 2), out_t("o_zgeo", 2)
    o_ztex, o_zn, o_znall = out_t("o_ztex", 2), out_t("o_zn", 2), out_t("o_znall", 6)
    o_kchart, o_kcode = out_t("o_kchart", 1, I32), out_t("o_kcode", 1, I32)
    o_indices, o_loss = out_t("o_indices", 3, I32), out_t("o_loss", 1)

    with tile.TileContext(nc) as tc, ExitStack() as ctx:
        def sb(name, p, f, dt=F32):
            return nc.alloc_sbuf_tensor(name, [p, f], dt).ap()

        c_s1, c_s2, c_s3 = sb("c_s1", 8, 128), sb("c_s2", 128, 128), sb("c_s3", 128, 20)
        c_b1, c_b2 = sb("c_b1", 128, 1), sb("c_b2", 128, 1)
        c_bias20, c_eye20 = sb("c_bias20", 20, 1), sb("c_eye20", 20, 20)
        c_cb0, c_cb1 = sb("c_cb0", 128, 63), sb("c_cb1", 128, 63)
        c_ri, c_c21 = sb("c_ri", 128, 63), sb("c_c21", 128, 3)
        c_cbg, c_chb3 = sb("c_cbg", 128, 126), sb("c_chb3", 128, CH_H)
        c_sfb1 = sb("c_sfb1", 128, 2)
        c_c012 = sb("c_c012", 128, 3)

        for dst, src in ((c_s1, s1), (c_s2, s2), (c_s3, s3), (c_b1, b1v),
                         (c_b2, b2v), (c_bias20, bias20), (c_eye20, eye20),
                         (c_cb0, cb0T), (c_cb1, cb1T), (c_ri, riota),
                         (c_c21, c21), (c_cbg, cbg), (c_chb3, chb3),
                         (c_sfb1, sfb1v), (c_c012, c012)):
            nc.sync.dma_start(dst[:], src[:])

        ptsT = sb("ptsT", 128, n_mega * 80)
        expb = sb("expb", 128, n_ch * 3)
        sumb, rcpb = sb("sumb", 128, n_ch), sb("rcpb", 128, n_ch)
        routb = sb("routb", 128, n_ch * 3)
        kchf = sb("kchf", 128, n_ch)
        cbarb, vlb = sb("cbarb", 128, n_ch * 2), sb("vlb", 128, n_ch * 2)
        dbb = sb("dbb", 128, n_ch * 2)
        deltab = sb("deltab", 128, n_ch * 6)
        lossb = sb("lossb", 128, n_ch)
        t_a, t_b, t_c = sb("t_a", 128, n_ch), sb("t_b", 128, n_ch), sb("t_c", 128, n_ch)

        p_mm1 = ctx.enter_context(tc.tile_pool(name="p_mm1", bufs=2, space="PSUM"))
        p_mm2 = ctx.enter_context(tc.tile_pool(name="p_mm2", bufs=2, space="PSUM"))
        p_mm3 = ctx.enter_context(tc.tile_pool(name="p_mm3", bufs=2, space="PSUM"))
        p_tr = ctx.enter_context(tc.tile_pool(name="p_tr", bufs=2, space="PSUM"))
        s_h = ctx.enter_context(tc.tile_pool(name="s_h", bufs=3))
        s_m3 = ctx.enter_context(tc.tile_pool(name="s_m3", bufs=2))
        s_dist = ctx.enter_context(tc.tile_pool(name="s_dist", bufs=1))
        s_sf = ctx.enter_context(tc.tile_pool(name="s_sf", bufs=2))

        # ============== Phase 1: feature MLP (gelu) ======================
        for m in range(n_mega):
            xt = s_h.tile([8, 512], F32, tag="xt")
            nc.sync.dma_start(xt[:], xT[m])

            ps1 = p_mm1.tile([128, 512], F32, tag="ps1")
            nc.tensor.matmul(ps1[:], c_s1[:], xt[:], start=True, stop=True)
            h1 = s_h.tile([128, 512], F32, tag="h1")
            nc.scalar.activation(h1[:], ps1[:], AF.Gelu, bias=c_b1[:, 0:1])

            ps2 = p_mm2.tile([128, 512], F32, tag="ps2")
            nc.tensor.matmul(ps2[:], c_s2[:], h1[:], start=True, stop=True)
            h2 = s_h.tile([128, 512], F32, tag="h2")
            nc.scalar.activation(h2[:], ps2[:], AF.Gelu, bias=c_b2[:, 0:1])

            ps3 = p_mm3.tile([20, 512], F32, tag="ps3")
            nc.tensor.matmul(ps3[:], c_s3[:], h2[:], start=True, stop=True)
            m3 = s_m3.tile([20, 512], F32, tag="m3")
            nc.vector.tensor_scalar(m3[:], ps3[:], c_bias20[:, 0:1], None, OP.add)

            ptr = p_tr.tile([128, 80], F32, tag="ptr")
            for b in range(4):
                nc.tensor.transpose(ptr[:, b * 20:(b + 1) * 20],
                                    m3[:, b * 128:(b + 1) * 128], c_eye20[:])
            # (b, g) -> (g, b) swap: chunk index becomes point-contiguous
            src = ptr[:].rearrange("p (b g t) -> p b g t", b=4, g=4, t=5)
            dst = ptsT[:, m * 80:(m + 1) * 80].rearrange(
                "p (g b t) -> p g b t", g=4, b=4, t=5).transpose((0, 2, 1, 3))
            nc.vector.tensor_copy(dst, src)

        # ============== Phase 2: routing + VQ (exp) ======================
        pts4 = ptsT[:].rearrange("p (ch t) -> p ch t", t=5)
        e3 = expb[:].rearrange("p (ch c) -> p ch c", c=3)
        nc.scalar.activation(e3, pts4[:, :, 2:5], AF.Exp)
        nc.vector.tensor_reduce(sumb[:], e3, AX.X, OP.add)
        nc.vector.reciprocal(rcpb[:], sumb[:])
        r3 = routb[:].rearrange("p (ch c) -> p ch c", c=3)
        nc.vector.tensor_tensor(
            r3, e3, rcpb[:].unsqueeze(2).broadcast_to((128, n_ch, 3)), OP.mult)

        # K_chart = argmax_c (first wins ties)
        nc.vector.tensor_tensor(t_a[:], e3[:, :, 0], e3[:, :, 1], OP.max)
        nc.vector.tensor_tensor(t_b[:], e3[:, :, 0], e3[:, :, 1], OP.is_lt)
        nc.vector.tensor_tensor(t_c[:], e3[:, :, 2], t_a[:], OP.is_gt)
        nc.vector.tensor_tensor(t_a[:], t_b[:], t_c[:], OP.mult)
        nc.vector.tensor_tensor(t_b[:], t_b[:], t_a[:], OP.subtract)
        nc.vector.scalar_tensor_tensor(kchf[:], t_c[:], 2.0, t_b[:], OP.mult, OP.add)

        cb3 = cbarb[:].rearrange("p (ch d) -> p ch d", d=2)
        for d in range(D):
            nc.vector.tensor_scalar(t_a[:], r3[:, :, 0], float(CMAT[0, d]),
                                    None, OP.mult)
            nc.vector.scalar_tensor_tensor(t_b[:], r3[:, :, 1], float(CMAT[1, d]),
                                           t_a[:], OP.mult, OP.add)
            nc.vector.scalar_tensor_tensor(cb3[:, :, d], r3[:, :, 2],
                                           float(CMAT[2, d]), t_b[:], OP.mult, OP.add)
        nc.vector.tensor_tensor(
            vlb[:].rearrange("p (ch d) -> p ch d", d=2), pts4[:, :, 0:2],
            cbarb[:].rearrange("p (ch d) -> p ch d", d=2), OP.subtract)

        vl3 = vlb[:].rearrange("p (ch d) -> p ch d", d=2)
        for (m0, nm) in _hypers(n_mega):
            ch0, chn = m0 * CH_MEGA, nm * CH_MEGA
            sl1, sl2 = ch0, ch0 + chn

            t0 = s_dist.tile([128, chn * 63], F32, tag="t0")
            t1 = s_dist.tile([128, chn * 63], F32, tag="t1")
            dt_ = s_dist.tile([128, chn * 63], F32, tag="dt")
            t0r = t0[:].rearrange("p (ch ck) -> p ch ck", ck=63)
            t1r = t1[:].rearrange("p (ch ck) -> p ch ck", ck=63)
            dtr = dt_[:].rearrange("p (ch ck) -> p ch ck", ck=63)
            vd0 = vl3[:, sl1:sl2, 0:1].broadcast_to((128, chn, 63))
            vd1 = vl3[:, sl1:sl2, 1:2].broadcast_to((128, chn, 63))
            cb0r = c_cb0[:].unsqueeze(1).broadcast_to((128, chn, 63))
            cb1r = c_cb1[:].unsqueeze(1).broadcast_to((128, chn, 63))
            nc.vector.tensor_tensor(t0r, vd0, cb0r, OP.subtract)
            nc.vector.tensor_tensor(t1r, vd1, cb1r, OP.subtract)
            nc.vector.tensor_tensor(t0r, t0r, t0r, OP.mult)
            nc.vector.scalar_tensor_tensor(dtr, t1r, 0.0, t1r, OP.add, OP.mult)
            nc.vector.tensor_tensor(dtr, dtr, t0r, OP.add)

            d4 = dt_[:].rearrange("p (ch c k) -> p ch c k", c=3, k=21)
            mrow = s_dist.tile([128, chn * 3], F32, tag="mrow")
            mrow3 = mrow[:].rearrange("p (ch c) -> p ch c", c=3)
            nc.vector.tensor_reduce(mrow3, d4, AX.X, OP.min)
            mrep = mrow3.unsqueeze(3).broadcast_to((128, chn, 3, 21))
            nc.vector.tensor_tensor(d4, d4, mrep, OP.is_le)  # onehot mask
            # idx = 20 - max_k(mask * (20-k))   (first-min wins ties)
            rir = c_ri[:].rearrange("p (c k) -> p c k", c=3).unsqueeze(1) \
                .broadcast_to((128, chn, 3, 21))
            t14 = t1[:].rearrange("p (ch c k) -> p ch c k", c=3, k=21)
            nc.vector.tensor_tensor(t14, d4, rir, OP.mult)
            idx_h = s_dist.tile([128, chn * 3], F32, tag="idx_h")
            idx3 = idx_h[:].rearrange("p (ch c) -> p ch c", c=3)
            nc.vector.tensor_reduce(idx3, t14, AX.X, OP.max)
            nc.vector.tensor_scalar(idx_h[:], idx_h[:], -1.0, 20.0, OP.mult, OP.add)
            idxi = s_dist.tile([128, chn * 3], I32, tag="idxi")
            nc.vector.tensor_copy(idxi[:], idx_h[:])
            nc.sync.dma_start(o_indices[:, sl1 * 3:sl2 * 3], idxi[:])

            # z_q = sum_k mask * cb  (exact gather; ties ~impossible in fp32)
            zq = s_dist.tile([128, chn * 6], F32, tag="zq")
            zq4 = zq[:].rearrange("p (ch c d) -> p ch c d", c=3, d=2)
            cb0s = c_cb0[:].rearrange("p (c k) -> p c k", c=3).unsqueeze(1) \
                .broadcast_to((128, chn, 3, 21))
            cb1s = c_cb1[:].rearrange("p (c k) -> p c k", c=3).unsqueeze(1) \
                .broadcast_to((128, chn, 3, 21))
            for d_i, cbs in ((0, cb0s), (1, cb1s)):
                nc.vector.tensor_tensor(t14, d4, cbs, OP.mult)
                nc.vector.tensor_reduce(
                    zq4[:, :, :, d_i:d_i + 1].squeeze(3), t14, AX.X, OP.add)

            # K_code = sum_c (c == K_chart) * idx_c
            oh3 = s_dist.tile([128, chn * 3], F32, tag="oh3")
            oh3r = oh3[:].rearrange("p (ch c) -> p ch c", c=3)
            c012r = c_c012[:].unsqueeze(1).broadcast_to((128, chn, 3))
            kchr = kchf[:, sl1:sl2].unsqueeze(2).broadcast_to((128, chn, 3))
            nc.vector.tensor_tensor(oh3r, c012r, kchr, OP.is_equal)
            nc.vector.tensor_tensor(oh3r, oh3r, idx3, OP.mult)
            kcf = s_dist.tile([128, chn], F32, tag="kcf")
            nc.vector.tensor_reduce(kcf[:], oh3r, AX.X, OP.add)
            kci = s_dist.tile([128, chn], I32, tag="kci")
            nc.vector.tensor_copy(kci[:], kcf[:])
            nc.sync.dma_start(o_kcode[:, sl1:sl2], kci[:])

            dl = deltab[:, sl1 * 6:sl2 * 6]
            dl4 = dl.rearrange("p (ch c d) -> p ch c d", c=3, d=2)
            vrep = vl3[:, sl1:sl2].unsqueeze(2).broadcast_to((128, chn, 3, 2))
            zq4 = zq[:].rearrange("p (ch c d) -> p ch c d", c=3, d=2)
            nc.vector.tensor_tensor(dl4, vrep, zq4, OP.subtract)

            sq = s_dist.tile([128, chn * 6], F32, tag="sq")
            nc.vector.tensor_tensor(sq[:], dl, dl, OP.mult)
            rrep = r3[:, sl1:sl2].unsqueeze(3).broadcast_to((128, chn, 3, 2))
            sq4 = sq[:].rearrange("p (ch c d) -> p ch c d", c=3, d=2)
            nc.vector.tensor_tensor(sq4, sq4, rrep, OP.mult)
            nc.vector.tensor_reduce(
                lossb[:, sl1:sl2],
                sq[:].rearrange("p (ch cd) -> p ch cd", cd=6), AX.X, OP.add)

            zqr = s_dist.tile([128, chn * 6], F32, tag="zqr")
            zqr4 = zqr[:].rearrange("p (ch c d) -> p ch c d", c=3, d=2)
            nc.vector.tensor_tensor(zqr4, zq4, rrep, OP.mult)
            zqbl = s_dist.tile([128, chn * 2], F32, tag="zqbl")
            nc.vector.tensor_reduce(
                zqbl[:].rearrange("p (ch d) -> p ch d", d=2),
                zqr4.transpose((0, 1, 3, 2)), AX.X, OP.add)
            zg = s_dist.tile([128, chn * 2], F32, tag="zg")
            nc.vector.tensor_tensor(zg[:], cbarb[:, sl1 * 2:sl2 * 2], zqbl[:],
                                    OP.add)
            nc.sync.dma_start(o_zgeo[:, sl1 * 2:sl2 * 2], zg[:])
            nc.vector.tensor_tensor(dbb[:, sl1 * 2:sl2 * 2],
                                    vlb[:, sl1 * 2:sl2 * 2], zqbl[:], OP.subtract)

        kchi = s_dist.tile([128, n_ch], I32, tag="kchi")
        nc.vector.tensor_copy(kchi[:], kchf[:])
        nc.sync.dma_start(o_kchart[:], kchi[:])
        nc.sync.dma_start(o_router[:], routb[:])
        nc.sync.dma_start(o_cbar[:], cbarb[:])
        nc.sync.dma_start(o_loss[:], lossb[:])

        # ============== Phase 3: structure filter (gelu) =================
        dall = deltab[:].rearrange("p (ch c d) -> p ch c d", c=3, d=2)
        for (m0, nm) in _hypers(n_mega):
            ch0, chn = m0 * CH_MEGA, nm * CH_MEGA
            sl1, sl2 = ch0, ch0 + chn
            d0 = dall[:, sl1:sl2, :, 0:1].squeeze(3)   # [128, chn, 3]
            d1 = dall[:, sl1:sl2, :, 1:2].squeeze(3)
            g0 = s_sf.tile([128, chn * 3], F32, tag="g0")
            g1 = s_sf.tile([128, chn * 3], F32, tag="g1")
            tt = s_sf.tile([128, chn * 3], F32, tag="tt")
            tt3 = tt[:].rearrange("p (ch c) -> p ch c", c=3)
            for e, ge in enumerate((g0, g1)):
                ge3 = ge[:].rearrange("p (ch c) -> p ch c", c=3)
                nc.vector.tensor_scalar(tt3, d0, float(SFW1[0, e]), None, OP.mult)
                nc.vector.scalar_tensor_tensor(tt3, d1, float(SFW1[1, e]),
                                               tt3, OP.mult, OP.add)
                nc.scalar.activation(ge3, tt3, AF.Gelu, bias=c_sfb1[:, e:e + 1])
            zna = s_sf.tile([128, chn * 6], F32, tag="zna")
            zna3 = zna[:].rearrange("p (cc f) -> p cc f", f=2)
            for f in range(2):
                nc.vector.tensor_scalar(tt[:], g0[:], float(SFW2[0, f]),
                                        float(SFB2[f]), OP.mult, OP.add)
                nc.vector.scalar_tensor_tensor(zna3[:, :, f:f + 1].squeeze(2),
                                               g1[:], float(SFW2[1, f]), tt[:],
                                               OP.mult, OP.add)
            nc.sync.dma_start(o_znall[:, sl1 * 6:sl2 * 6], zna[:])

            znw = s_sf.tile([128, chn * 6], F32, tag="znw")
            znw4 = znw[:].rearrange("p (ch c f) -> p ch c f", c=3, f=2)
            rrep = r3[:, sl1:sl2].unsqueeze(3).broadcast_to((128, chn, 3, 2))
            nc.vector.tensor_tensor(
                znw4, zna[:].rearrange("p (ch c f) -> p ch c f", c=3, f=2),
                rrep, OP.mult)
            znh = s_sf.tile([128, chn * 2], F32, tag="znh")
            nc.vector.tensor_reduce(
                znh[:].rearrange("p (ch f) -> p ch f", f=2),
                znw4.transpose((0, 1, 3, 2)), AX.X, OP.add)
            nc.sync.dma_start(o_zn[:, sl1 * 2:sl2 * 2], znh[:])
            ztx = s_sf.tile([128, chn * 2], F32, tag="ztx")
            nc.vector.tensor_tensor(ztx[:], dbb[:, sl1 * 2:sl2 * 2], znh[:],
                                    OP.subtract)
            nc.sync.dma_start(o_ztex[:, sl1 * 2:sl2 * 2], ztx[:])

    nc.compile()
    return nc


# ========================== host side ====================================

def _prep_consts(inputs):
    f = np.float32
    w1 = np.asarray(inputs["feat_w1"], f)
    w2 = np.asarray(inputs["feat_w2"], f)
    w3 = np.asarray(inputs["val_w"], f)
    b1 = np.asarray(inputs["feat_b1"], f)
    b2 = np.asarray(inputs["feat_b2"], f)
    b3 = np.asarray(inputs["val_b"], f)
    C = np.asarray(inputs["chart_centers"], f)
    cb = np.asarray(inputs["codebook"], f)
    isq = np.float32(1.0 / np.sqrt(np.float32(D)))

    s1 = np.zeros((8, 128), f)
    s2 = np.zeros((128, 128), f)
    for g in range(4):
        for d in range(IN):
            s1[4 * d + g, 32 * g:32 * (g + 1)] = w1[d]
        s2[32 * g:32 * (g + 1), 32 * g:32 * (g + 1)] = w2
    w3c = np.concatenate([w3, (w3 @ C.T) * isq], axis=1)
    s3 = np.zeros((128, 20), f)
    for g in range(4):
        s3[32 * g:32 * (g + 1), 5 * g:5 * (g + 1)] = w3c
    bias5 = np.concatenate([b3, (b3 @ C.T) * isq]).astype(f)
    cbf = cb.reshape(63, 2)
    return dict(
        s1=s1, s2=s2, s3=s3,
        b1v=np.tile(b1, 4).reshape(128, 1).astype(f),
        b2v=np.tile(b2, 4).reshape(128, 1).astype(f),
        bias20=np.tile(bias5, 4).reshape(20, 1).astype(f),
        eye20=np.eye(20, dtype=f),
        cb0T=np.tile(cbf[:, 0], (128, 1)).astype(f),
        cb1T=np.tile(cbf[:, 1], (128, 1)).astype(f),
        riota=np.tile(np.tile((20 - np.arange(21)).astype(f), 3), (128, 1)),
        c21=np.tile((21.0 * np.arange(3)).astype(f), (128, 1)),
        cbg=np.tile(cbf.reshape(-1), (128, 1)).astype(f),
        chb3=np.tile((3.0 * np.arange(CH_H)).astype(f), (128, 1)),
        sfb1v=np.tile(np.asarray(inputs['sf_b1'], f), (128, 1)),
        c012=np.tile(np.arange(3).astype(f), (128, 1)),
    )


def _prep_xt(x, core, n_mega):
    f = np.float32
    p_core = n_mega * MT
    xs = x[core * B_CORE:(core + 1) * B_CORE]
    if xs.shape[0] < p_core:
        xs = np.pad(xs, ((0, p_core - xs.shape[0]), (0, 0)))
    else:
        xs = xs[:p_core]
    # [n_mega, 4g, 512, 2] -> rows 4d+g
    blk = xs.reshape(n_mega, 4, 512, IN)
    xt = np.empty((n_mega, 8, 512), f)
    for d in range(IN):
        xt[:, 4 * d:4 * (d + 1), :] = blk[:, :, :, d]
    return xt


def _unperm(a, w, n_pts):
    n_ch2 = a.shape[1] // w
    return np.ascontiguousarray(
        a.reshape(128, n_ch2, w).transpose(1, 0, 2).reshape(-1, w)[:n_pts])


_NC_CACHE = {}


def _get_nc(inputs, n_mega):
    f = np.float32
    key = n_mega
    if key not in _NC_CACHE:
        wts = dict(
            CMAT=np.asarray(inputs["chart_centers"], f),
            SFW1=np.asarray(inputs["sf_w1"], f),
            SFW2=np.asarray(inputs["sf_w2"], f),
            SFB1=np.asarray(inputs["sf_b1"], f),
            SFB2=np.asarray(inputs["sf_b2"], f),
        )
        _NC_CACHE[key] = build_kernel(wts, n_mega)
    return _NC_CACHE[key]


def make_in_maps(inputs, n_mega):
    consts = _prep_consts(inputs)
    x = np.asarray(inputs["x"], np.float32)
    in_maps = []
    for c in range(N_CORES):
        im = dict(consts)
        im["xT"] = _prep_xt(x, c, n_mega)
        in_maps.append(im)
    return in_maps


def kernel(**inputs):
    n_mega = (B_CORE + MT - 1) // MT
    nc = _get_nc(inputs, n_mega)
    in_maps = make_in_maps(inputs, n_mega)
    res = run_bass_kernel_spmd(nc, in_maps, list(range(N_CORES))).results
    return postprocess(res)


def postprocess(res):
    outs = {k: [] for k in ("K_chart", "K_code", "z_n", "z_tex", "router",
                            "z_geo", "indices", "z_n_all", "c_bar")}
    loss_sum = np.float64(0.0)
    for c in range(N_CORES):
        r = res[c]
        outs["K_chart"].append(_unperm(r["o_kchart"], 1, B_CORE)[:, 0])
        outs["K_code"].append(_unperm(r["o_kcode"], 1, B_CORE)[:, 0])
        outs["z_n"].append(_unperm(r["o_zn"], 2, B_CORE))
        outs["z_tex"].append(_unperm(r["o_ztex"], 2, B_CORE))
        outs["router"].append(_unperm(r["o_router"], 3, B_CORE))
        outs["z_geo"].append(_unperm(r["o_zgeo"], 2, B_CORE))
        outs["indices"].append(_unperm(r["o_indices"], 3, B_CORE))
        outs["z_n_all"].append(_unperm(r["o_znall"], 6, B_CORE).reshape(-1, 3, 2))
        outs["c_bar"].append(_unperm(r["o_cbar"], 2, B_CORE))
        loss_sum += _unperm(r["o_loss"], 1, B_CORE).sum(dtype=np.float64)

    cat = {k: np.concatenate(v) for k, v in outs.items()}
    vq_loss = np.float32(1.25 * loss_sum / (2.0 * B))
    return (np.ascontiguousarray(cat["K_chart"]).astype(np.int32),
            np.ascontiguousarray(cat["K_code"]).astype(np.int32),
            cat["z_n"], cat["z_tex"], cat["router"], cat["z_geo"], vq_loss,
            cat["indices"].astype(np.int32), cat["z_n_all"], cat["c_bar"])
